# revision 6
# baseline (speedup 1.0000x reference)
"""Trainium2 Bass kernel for nn_BatchMAFLayer (v2).

Per-core: 8 MADEs x full batch B=4096. Host pre-applies masks, converts
dtypes, and lays weights out DMA-ready (no device-side weight prep).

Device pipeline per 512-col batch chunk:
  L1  bf16 row-tiled matmuls (K=32 at row group 32i) -> PSUM [128,1024]
  relu/cast -> h1 fp8e4 in SBUF (ACT/DVE split for balance)
  L2  fp8 DoubleRow matmuls: K=256 in ONE instruction per output half
  relu/cast -> h2 bf16
  L3  bf16 col-tiled M=32 matmuls into po3 [128,2048] (4 pairs side by
      side in the free dim), with the x subtraction FOLDED INTO the
      accumulation group via a [-I;-I] stationary on a bf16 hi/lo
      split of x (exact to ~2^-16), so po3 rows 0:64 hold shift-x.
  tail (one op per algebraic step at FD=2048):
      e2 = exp(-2*po3[ls])     (ACT)
      sq = square(po3[shift])  (ACT)      # (shift-x)^2
      z[64:128] = po3[ls] + C  (ACT copy) # C = 0.5*log(2*pi)
      z[0:64]   = sq * e2      (GpSimd)   # y^2
  LL  f16 d-reduction matmuls with a zero-padded coeff stack
      (-0.5 on y^2 rows, -1 on ls rows) accumulating all 4 pairs into
      pll aliased at po3[0:8, 0:512] (safe: all tail reads of po3 are
      complete once z is written).

PSUM budget: psum_h [128,1024] bufs=2 (4 banks) + po3 [128,2048]
bufs=1 (4 banks); pll aliased inside po3.
"""

import numpy as np
import ml_dtypes

import concourse.bass as bass
from concourse import bacc
import concourse.mybir as mybir
import concourse.tile as tile

F32 = mybir.dt.float32
BF16 = mybir.dt.bfloat16
F16 = mybir.dt.float16
FP8 = mybir.dt.float8e4
AFT = mybir.ActivationFunctionType
DR = mybir.MatmulPerfMode.DoubleRow

D = 32
N_BATCH = 64
HID = 256
B = 4096
NCORES = 8
NPC = N_BATCH // NCORES
CH = 512
NCH = B // CH
HALF_LOG_2PI = 0.5 * float(np.log(2.0 * np.pi))

# relu drain assignment: which of the 16 per-chunk units go to ACT
# (u = 4*pair + {0:h1A, 1:h2A, 2:h1B, 3:h2B}); rest go to DVE.
ACT_UNITS = frozenset((0, 3, 5, 8, 11, 14))


def build_nc():
    nc = bacc.Bacc("TRN2", target_bir_lowering=False)

    xq_d = nc.dram_tensor("xq", [D, B], BF16, kind="ExternalInput")
    xhl_d = nc.dram_tensor("xhl", [2 * D, B], BF16, kind="ExternalInput")
    negI_d = nc.dram_tensor("negI", [2 * D, 2 * D], BF16, kind="ExternalInput")
    w1_d = nc.dram_tensor("w1", [NPC // 4, 128, HID], BF16, kind="ExternalInput")
    w2_d = nc.dram_tensor("w2", [NPC, 128, 2 * HID], FP8, kind="ExternalInput")
    w3_d = nc.dram_tensor("w3", [NPC, 128, 4 * D], BF16, kind="ExternalInput")
    coeff_d = nc.dram_tensor("coeff", [128, NPC * (NPC // 2)], F16,
                             kind="ExternalInput")
    out_d = nc.dram_tensor("out", [NPC, B], F32, kind="ExternalOutput")

    with tile.TileContext(nc) as tc:
        with (
            tc.tile_pool(name="persist", bufs=1) as persist,
            tc.tile_pool(name="h1p", bufs=3) as h1p,
            tc.tile_pool(name="h2p", bufs=10) as h2p,
            tc.tile_pool(name="tailp", bufs=2) as tailp,
            tc.tile_pool(name="psum_h", bufs=2, space="PSUM") as psum_h,
            tc.tile_pool(name="psum_po", bufs=1, space="PSUM") as psum_po,
        ):
            # x (bf16 hi) replicated to all four 32-partition row groups
            xq = persist.tile([128, B], BF16, tag="xq")
            for r in range(4):
                nc.sync.dma_start(out=xq[32 * r: 32 * r + 32, :], in_=xq_d[:, :])
            xhl = persist.tile([2 * D, B], BF16, tag="xhl")
            nc.sync.dma_start(out=xhl, in_=xhl_d[:, :])
            negI = persist.tile([2 * D, 2 * D], BF16, tag="negI")
            nc.sync.dma_start(out=negI, in_=negI_d[:, :])

            W1 = []
            for g in range(NPC // 4):
                t = persist.tile([128, HID], BF16, tag=f"w1_{g}")
                nc.sync.dma_start(out=t, in_=w1_d[g])
                W1.append(t)
            W2 = []
            for n in range(NPC):
                t = persist.tile([128, 2 * HID], FP8, tag=f"w2_{n}")
                nc.sync.dma_start(out=t, in_=w2_d[n])
                W2.append(t)
            W3 = []
            for n in range(NPC):
                t = persist.tile([128, 4 * D], BF16, tag=f"w3_{n}")
                nc.sync.dma_start(out=t, in_=w3_d[n])
                W3.append(t)
            coeff = persist.tile([128, NPC * (NPC // 2)], F16, tag="coeff")
            nc.sync.dma_start(out=coeff, in_=coeff_d[:, :])

            def emit_ll(c_prev, z_prev, po3_prev):
                """d-reduction for chunk c_prev: all 4 pairs accumulate into
                pll aliased at po3_prev[0:8, 0:512] (all tail reads of
                po3_prev are complete once z_prev is fully written)."""
                pll = po3_prev[0:NPC, 0:CH]
                for j in range(NPC // 2):
                    nc.tensor.matmul(
                        pll, coeff[:, NPC * j: NPC * (j + 1)],
                        z_prev[:, CH * j: CH * (j + 1)],
                        start=(j == 0), stop=(j == NPC // 2 - 1),
                        skip_group_check=True,
                    )
                llt = tailp.tile([NPC, CH], F32, tag="llt")
                nc.vector.tensor_copy(llt, pll)
                cps = slice(c_prev * CH, (c_prev + 1) * CH)
                nc.sync.dma_start(out=out_d[:, cps], in_=llt)

            pending = None  # (c_prev, z_prev, po3_prev)
            for c in range(NCH):
                cs = slice(c * CH, (c + 1) * CH)

                # ---- phase A: L1 + L2 for all 8 mades --------------------
                h2s = []
                for n in range(NPC):
                    g, i = n // 4, n % 4
                    rs = slice(32 * i, 32 * i + 32)

                    ph1 = psum_h.tile([128, 2 * CH], F32, tag="ph")
                    for mo in range(2):
                        nc.tensor.matmul(
                            ph1[:, CH * mo: CH * (mo + 1)],
                            W1[g][rs, 128 * mo: 128 * (mo + 1)],
                            xq[rs, cs],
                            start=True, stop=True,
                            tile_position=(32 * i, 0),
                        )
                    h1 = h1p.tile([128, 2 * CH], FP8, tag="h1")
                    if 2 * n in ACT_UNITS:
                        nc.scalar.activation(h1, ph1, AFT.Relu)
                    else:
                        nc.vector.tensor_scalar_max(h1, ph1, 0.0)

                    ph2 = psum_h.tile([128, 2 * CH], F32, tag="ph")
                    for mo in range(2):
                        for t in range(2):
                            nc.tensor.matmul(
                                ph2[:, CH * mo: CH * (mo + 1)],
                                W2[n][:, HID * mo + 128 * t:
                                      HID * mo + 128 * (t + 1)],
                                h1[:, CH * t: CH * (t + 1)],
                                start=(t == 0), stop=(t == 1),
                            )
                    h2 = h2p.tile([128, 2 * CH], BF16, tag="h2")
                    if 2 * n + 1 in ACT_UNITS:
                        nc.scalar.activation(h2, ph2, AFT.Relu)
                    else:
                        nc.vector.tensor_scalar_max(h2, ph2, 0.0)
                    h2s.append(h2)

                # ---- phase B: deferred d-reduction of the previous chunk -
                # (its z has had a full phase of slack; po3 is then free)
                if pending is not None:
                    emit_ll(*pending)

                # ---- phase C: x-fold + L3 into po3 -----------------------
                po3 = psum_po.tile([128, 4 * CH], F32, tag="po3")
                for j in range(NPC // 2):
                    pcs = slice(CH * j, CH * (j + 1))
                    nc.tensor.matmul(
                        po3[0:64, pcs], negI, xhl[:, cs],
                        start=True, stop=False, skip_group_check=True,
                        tile_position=(0, 0),
                    )
                    for kh in range(2):
                        for h2t, w3t, cg in (
                            (h2s[2 * j], W3[2 * j], 0),
                            (h2s[2 * j + 1], W3[2 * j + 1], 32),
                        ):
                            rhs = h2t[:, CH * kh: CH * (kh + 1)]
                            nc.tensor.matmul(
                                po3[cg: cg + 32, pcs],
                                w3t[:, 64 * kh: 64 * kh + 32],
                                rhs,
                                start=False, stop=(kh == 1),
                                skip_group_check=True,
                                tile_position=(0, cg),
                            )
                            nc.tensor.matmul(
                                po3[64 + cg: 96 + cg, pcs],
                                w3t[:, 64 * kh + 32: 64 * kh + 64],
                                rhs,
                                start=(kh == 0), stop=(kh == 1),
                                skip_group_check=True,
                                tile_position=(0, 64 + cg),
                            )

                # ---- phase D: tail on the merged po3 (FD = 2048) ---------
                z = tailp.tile([128, 4 * CH], F16, tag="z")
                e2 = tailp.tile([64, 4 * CH], F16, tag="e2")
                nc.scalar.activation(e2, po3[64:128, :], AFT.Exp, scale=-2.0)
                sq = tailp.tile([64, 4 * CH], F16, tag="sq")
                nc.scalar.activation(sq, po3[0:64, :], AFT.Square)
                nc.scalar.activation(z[64:128, :], po3[64:128, :], AFT.Copy,
                                     bias=HALF_LOG_2PI)
                nc.gpsimd.tensor_mul(z[0:64, :], sq, e2)
                pending = (c, z, po3)

            emit_ll(*pending)

    nc.compile()
    return nc


_NC_CACHE = None
RUN_KWARGS = {}
LAST_RESULT = None


def _get_nc():
    global _NC_CACHE
    if _NC_CACHE is None:
        _NC_CACHE = build_nc()
    return _NC_CACHE


def _prep_core(W1m, W2m, W3m):
    """Per-core weight layouts from masked f32 weights.

    W1m [8, 32, 256], W2m [8, 256, 256], W3m [8, 256, 64] ->
      w1 [2, 128, 256] bf16   (made 4g+i at partitions 32i)
      w2 [8, 128, 512] fp8e4  (col = 256*mo + 128*t + m; row p = k - 128t)
      w3 [8, 128, 128] bf16   (col = 64*kh + 32*param + d)
    """
    w1 = np.zeros((2, 128, HID), dtype=np.float32)
    for n in range(NPC):
        g, i = n // 4, n % 4
        w1[g, 32 * i: 32 * i + 32, :] = W1m[n]
    w2 = np.zeros((NPC, 128, 2 * HID), dtype=np.float32)
    for n in range(NPC):
        for mo in range(2):
            for t in range(2):
                w2[n, :, 256 * mo + 128 * t: 256 * mo + 128 * (t + 1)] = \
                    W2m[n, 128 * t: 128 * (t + 1), 128 * mo: 128 * (mo + 1)]
    w3 = np.zeros((NPC, 128, 4 * D), dtype=np.float32)
    for n in range(NPC):
        for kh in range(2):
            blk = W3m[n, 128 * kh: 128 * (kh + 1), :]  # [128, 64] interleaved
            w3[n, :, 64 * kh: 64 * kh + 32] = blk[:, 0::2]   # shift
            w3[n, :, 64 * kh + 32: 64 * kh + 64] = blk[:, 1::2]  # log-scale
    return (
        w1.astype(ml_dtypes.bfloat16),
        w2.astype(ml_dtypes.float8_e4m3),
        w3.astype(ml_dtypes.bfloat16),
    )


def _prep_shared(x):
    xr = np.asarray(x[:, :D], dtype=np.float32)
    xT = np.ascontiguousarray(xr.T)               # [32, B]
    x_hi = xT.astype(ml_dtypes.bfloat16)
    x_lo = (xT - x_hi.astype(np.float32)).astype(ml_dtypes.bfloat16)
    xhl = np.concatenate([x_hi, x_lo], axis=0)    # [64, B]

    negI = -np.tile(np.eye(D, dtype=np.float32), (2, 2))
    negI = negI.astype(ml_dtypes.bfloat16)        # [64, 64]

    coeff = np.zeros((128, NPC * (NPC // 2)), dtype=np.float32)
    for j in range(NPC // 2):
        for p in range(2):
            col = NPC * j + 2 * j + p
            coeff[32 * p: 32 * p + 32, col] = -0.5
            coeff[64 + 32 * p: 96 + 32 * p, col] = -1.0
    coeff = coeff.astype(np.float16)
    return x_hi, xhl, negI, coeff


def kernel(x, w1, w2, w3, m1, m2, m3):
    from concourse.bass_utils import run_bass_kernel_spmd

    x = np.asarray(x, dtype=np.float32)
    W1m = np.asarray(w1, dtype=np.float32) * np.asarray(m1).astype(np.float32)
    W2m = np.asarray(w2, dtype=np.float32) * np.asarray(m2).astype(np.float32)
    W3m = np.asarray(w3, dtype=np.float32) * np.asarray(m3).astype(np.float32)

    x_hi, xhl, negI, coeff = _prep_shared(x)

    in_maps = []
    for k in range(NCORES):
        s = slice(k * NPC, (k + 1) * NPC)
        w1t, w2t, w3t = _prep_core(W1m[s], W2m[s], W3m[s])
        in_maps.append(
            {
                "xq": x_hi,
                "xhl": xhl,
                "negI": negI,
                "w1": w1t,
                "w2": w2t,
                "w3": w3t,
                "coeff": coeff,
            }
        )

    nc = _get_nc()
    res = run_bass_kernel_spmd(nc, in_maps, list(range(NCORES)), **RUN_KWARGS)
    global LAST_RESULT
    LAST_RESULT = res
    results = res.results
    return np.concatenate([results[k]["out"].T for k in range(NCORES)], axis=1)
